# revision 1
# baseline (speedup 1.0000x reference)
"""MeanField CRF message-passing kernel for 8 Trainium2 NeuronCores.

Sharding: (B=2) x (H into 4 chunks of 128 rows) = 8 slabs, each with a
5-row halo on slab-interior edges (5 mean-field iterations x 1-row
stencil reach), so cores run fully independently (no collectives).

Per-core layouts:
  x-layout  : [x mod 128 -> partitions, (xblock, y, class) -> free]
  C-packed  : [(6 rows x 21 classes)=126 -> partitions, x -> free]
Math per iteration (equivalent-transformed from the reference):
  E   = exp(-Y) in-place on Y          (ACT, x-layout; Y starts as u)
  s   = sum_c E ; r = 1/s              (gpsimd reduce + DVE reciprocal)
  m   = blockdiag(LC^T/8) @ E_C        (PE matmul, C-packed via PE transposes)
  w2_d= ew_d * shift_d(r)              (normalizer folded into weights)
  Y   = u + sum_d w2_d * shift_d(m)    (DVE multiply/accumulate, x-layout)
Final cost = Y after iteration 5 (no softmax on the last iteration).
"""

import sys

sys.path.insert(0, "/opt/trn_rl_repo")

import numpy as np

import concourse.bass as bass
import concourse.bacc as bacc
import concourse.tile as tile
from concourse import mybir
from concourse.bass_utils import run_bass_kernel_spmd

F32 = mybir.dt.float32
F16 = mybir.dt.float16
BF16 = mybir.dt.bfloat16

P = 128          # partitions
C = 21           # classes
RG = 6           # y-rows per C-packed group (6*21=126 partitions)
NB = 23          # row-blocks per slab (138 = 6*23)
YT = 138         # slab rows (128 own + 2*5 halo)
XB = 4           # x blocks (512 = 4*128)
D = 8            # directions
W = 512
HALO = 5
OWN = 128
MAX_ITER = 5
NSPAN = 8        # ceil(138/18)
DIRS = [(0, 1), (0, -1), (1, 0), (-1, 0), (1, 1), (1, -1), (-1, 1), (-1, -1)]

_CACHED_NC = None


def build_nc():
    nc = bacc.Bacc("TRN2")
    uu_d = nc.dram_tensor("uu", [P, XB, YT, C], F32, kind="ExternalInput")
    ew_d = nc.dram_tensor("ew", [P, D, XB, YT], F16, kind="ExternalInput")
    lcb_d = nc.dram_tensor("lcblk", [RG * C, RG * C], F32, kind="ExternalInput")
    ide_d = nc.dram_tensor("ident", [P, P], F32, kind="ExternalInput")
    yout_d = nc.dram_tensor("yout", [P, XB, YT, C], F32, kind="ExternalOutput")

    MUL = mybir.AluOpType.mult
    ADD = mybir.AluOpType.add

    with tile.TileContext(nc) as tc:
        with (
            tc.tile_pool(name="state", bufs=1) as st,
            tc.tile_pool(name="ecp", bufs=3) as ecp,
            tc.tile_pool(name="tp", bufs=5) as tp,
        ):
            UU = st.tile([P, XB, YT, C], F32)
            EWs = st.tile([P, D, XB, YT], F16)
            Y = st.tile([P, XB, YT, C], F32)
            MC = st.tile([RG * C, NB + 2, W + 2], F32)
            W2 = st.tile([P, D, XB, YT], F32)
            S0 = st.tile([P, XB, YT], F32)      # softmax sums -> reciprocal r
            RP = st.tile([P, XB, YT], F32)      # r shifted x+1
            RM = st.tile([P, XB, YT], F32)      # r shifted x-1
            IDE = st.tile([P, P], F32)
            LCB = st.tile([RG * C, RG * C], F32)

            nc.sync.dma_start(out=UU[:], in_=uu_d[:])
            nc.sync.dma_start(out=EWs[:], in_=ew_d[:])
            nc.sync.dma_start(out=IDE[:], in_=ide_d[:])
            nc.sync.dma_start(out=LCB[:], in_=lcb_d[:])
            nc.vector.memset(MC[:], 0)
            nc.vector.memset(W2[:], 0)
            nc.vector.memset(RP[:], 0)
            nc.vector.memset(RM[:], 0)

            for it in range(MAX_ITER):
                # ---- E = exp(-Y) (in-place; Y holds E afterwards) --------
                nc.scalar.activation(
                    out=Y[:], in_=(UU[:] if it == 0 else Y[:]),
                    func=mybir.ActivationFunctionType.Exp, scale=-1.0,
                )
                # ---- softmax sums + reciprocal + x-shifted copies --------
                nc.vector.tensor_reduce(
                    out=S0[:], in_=Y[:], op=ADD, axis=mybir.AxisListType.X,
                )
                nc.vector.reciprocal(out=S0[:], in_=S0[:])
                nc.sync.dma_start(out=RP[0 : P - 1, :, :], in_=S0[1:P, :, :])
                nc.sync.dma_start(
                    out=RP[P - 1 : P, 0 : XB - 1, :], in_=S0[0:1, 1:XB, :]
                )
                nc.sync.dma_start(out=RM[1:P, :, :], in_=S0[0 : P - 1, :, :])
                nc.sync.dma_start(
                    out=RM[0:1, 1:XB, :], in_=S0[P - 1 : P, 0 : XB - 1, :]
                )
                # ---- w2_d = ew_d * shift_d(r) ----------------------------
                for d, (dy, dx) in enumerate(DIRS):
                    rsrc = {-1: RM, 0: S0, 1: RP}[dx]
                    ylo, yhi = max(0, -dy), min(YT, YT - dy)
                    for xb in range(XB):
                        nc.vector.tensor_mul(
                            out=W2[:, d, xb, ylo:yhi],
                            in0=EWs[:, d, xb, ylo:yhi],
                            in1=rsrc[:, xb, ylo + dy : yhi + dy],
                        )

                # ---- phase A: T1 transposes + evac + LC matmul -----------
                # rb processed in pairs sharing 2-bank PSUM tiles so each
                # ACT evacuation copy amortizes its fixed overhead.
                with (
                    tc.tile_pool(name="pa", bufs=2, space="PSUM") as pa,
                    tc.tile_pool(name="pm", bufs=2, space="PSUM") as pm,
                ):
                    for rb0 in range(0, NB, 2):
                        pair = [rb for rb in (rb0, rb0 + 1) if rb < NB]
                        np_ = len(pair)
                        yc = pa.tile([RG * C, 2 * W], F32, tag="yc")
                        for k, rb in enumerate(pair):
                            for xb in range(XB):
                                blk = Y[:, xb, rb * RG : (rb + 1) * RG, :]
                                blk = blk.rearrange("p a b -> p (a b)")
                                nc.tensor.transpose(
                                    out=yc[:, k * W + xb * P : k * W + (xb + 1) * P],
                                    in_=blk,
                                    identity=IDE[:],
                                )
                        ec = ecp.tile([RG * C, 2 * W], F32, tag="ec")
                        nc.scalar.copy(
                            out=ec[:, 0 : np_ * W], in_=yc[:, 0 : np_ * W]
                        )
                        mcp = pm.tile([RG * C, 2 * W], F32, tag="mcp")
                        for k, rb in enumerate(pair):
                            nc.tensor.matmul(
                                out=mcp[:, k * W : (k + 1) * W],
                                lhsT=LCB[:],
                                rhs=ec[:, k * W : (k + 1) * W],
                                start=True, stop=True,
                            )
                        nc.scalar.copy(
                            out=MC[:, rb0 + 1 : rb0 + 1 + np_, 1 : W + 1],
                            in_=mcp[:, 0 : np_ * W].rearrange(
                                "p (a b) -> p a b", b=W
                            ),
                        )

                # ---- phase B: per-span shift transposes + aggregation ----
                with tc.tile_pool(name="pb", bufs=2, space="PSUM") as pb:
                    for xb in range(XB):
                        prev_mx = None
                        for s in range(NSPAN):
                            y0 = 18 * s
                            n = min(18, YT - y0)
                            nrb = (n + RG - 1) // RG  # 3, or 2 for last span
                            mx = {}
                            for dx in (-1, 0, 1):
                                t_mx = pb.tile(
                                    [P, 3 * RG * C], F32, tag=f"mx{dx + 1}"
                                )
                                for j in range(nrb):
                                    rbg = 3 * s + j
                                    a = xb * P + dx + 1
                                    nc.tensor.transpose(
                                        out=t_mx[:, j * RG * C : (j + 1) * RG * C],
                                        in_=MC[:, 1 + rbg, a : a + P],
                                        identity=IDE[: RG * C, : RG * C],
                                    )
                                mx[dx] = t_mx
                            # main accumulation: multiply into per-dy-group
                            # term tiles (bf16 except last iteration), tree-
                            # add within each group at 2x DVE rate, then one
                            # mixed add per group into fp32 Y.
                            tdt = F32 if it == MAX_ITER - 1 else BF16
                            first = True
                            for dy in (0, 1, -1):
                                dirs_g = [
                                    (d, dxx)
                                    for d, (dyy, dxx) in enumerate(DIRS)
                                    if dyy == dy
                                ]
                                a = max(y0, y0 - dy, -dy)
                                b = min(y0 + n, y0 + 18 - dy, YT - dy)
                                if b <= a:
                                    continue
                                cnt = b - a
                                off = (a + dy - y0) * C
                                ts = []
                                for d, dx in dirs_g:
                                    t_t = tp.tile([P, 18 * C], tdt, tag="tt")
                                    w2b = W2[:, d, xb, a:b].to_broadcast(
                                        (P, cnt, C)
                                    )
                                    m_in = mx[dx][
                                        :, off : off + cnt * C
                                    ].rearrange("p (a b) -> p a b", b=C)
                                    nc.vector.tensor_mul(
                                        out=t_t[:, 0 : cnt * C].rearrange(
                                            "p (a b) -> p a b", b=C
                                        ),
                                        in0=m_in,
                                        in1=w2b,
                                    )
                                    ts.append(t_t)
                                g = ts[0]
                                for t_n in ts[1:]:
                                    nc.vector.tensor_add(
                                        out=g[:, 0 : cnt * C],
                                        in0=g[:, 0 : cnt * C],
                                        in1=t_n[:, 0 : cnt * C],
                                    )
                                gr = g[:, 0 : cnt * C].rearrange(
                                    "p (a b) -> p a b", b=C
                                )
                                nc.vector.tensor_add(
                                    out=Y[:, xb, a:b, :],
                                    in0=(UU if first else Y)[:, xb, a:b, :],
                                    in1=gr,
                                )
                                first = False
                            # edge rows crossing span windows
                            if s >= 1:
                                for d, (dy, dx) in enumerate(DIRS):
                                    if dy == -1:
                                        ye = y0
                                        nc.vector.scalar_tensor_tensor(
                                            out=Y[:, xb, ye, :],
                                            in0=prev_mx[dx][:, 17 * C : 18 * C],
                                            scalar=W2[:, d, xb, ye : ye + 1],
                                            in1=Y[:, xb, ye, :],
                                            op0=MUL, op1=ADD,
                                        )
                                    elif dy == 1:
                                        ye = y0 - 1
                                        nc.vector.scalar_tensor_tensor(
                                            out=Y[:, xb, ye, :],
                                            in0=mx[dx][:, 0:C],
                                            scalar=W2[:, d, xb, ye : ye + 1],
                                            in1=Y[:, xb, ye, :],
                                            op0=MUL, op1=ADD,
                                        )
                            prev_mx = mx

            nc.sync.dma_start(out=yout_d[:], in_=Y[:])

    nc.finalize()
    return nc


def _prep_core(u, ew, b, hc):
    y0 = 128 * hc
    ys = min(max(y0 - HALO, 0), 512 - YT)
    u_slab = u[b, 0, :, ys : ys + YT, :]          # [21, 138, 512]
    ew_slab = ew[b, :, ys : ys + YT, :]           # [8, 138, 512]
    uu = np.ascontiguousarray(
        u_slab.reshape(C, YT, XB, P).transpose(3, 2, 1, 0), dtype=np.float32
    )
    ewp = np.ascontiguousarray(
        ew_slab.reshape(D, YT, XB, P).transpose(3, 0, 2, 1), dtype=np.float16
    )
    return uu, ewp, ys, y0 - ys


def kernel(unary, edge_weights, label_context, _trace=False, _tmpdir=None):
    global _CACHED_NC
    if _CACHED_NC is None:
        _CACHED_NC = build_nc()
    nc = _CACHED_NC

    u = np.asarray(unary, dtype=np.float32)
    ew = np.asarray(edge_weights, dtype=np.float32)
    lc = np.asarray(label_context, dtype=np.float32)

    lcblk = np.kron(np.eye(RG, dtype=np.float32), (lc.T / 8.0)).astype(np.float32)
    ident = np.eye(P, dtype=np.float32)

    in_maps = []
    offs = []
    for core in range(8):
        b, hc = core // 4, core % 4
        uu, ewp, ys, off = _prep_core(u, ew, b, hc)
        offs.append(off)
        in_maps.append({"uu": uu, "ew": ewp, "lcblk": lcblk, "ident": ident})

    kwargs = {}
    if _trace:
        kwargs = dict(trace=True, trace_cores=[0], tmpdir=_tmpdir)
    res = run_bass_kernel_spmd(nc, in_maps, core_ids=list(range(8)), **kwargs)

    out = np.zeros((2, 1, C, 512, 512), dtype=np.float32)
    for core in range(8):
        b, hc = core // 4, core % 4
        yo = res.results[core]["yout"]            # [P, XB, YT, C]
        slab = yo.transpose(3, 2, 1, 0).reshape(C, YT, W)
        off = offs[core]
        out[b, 0, :, 128 * hc : 128 * (hc + 1), :] = slab[:, off : off + OWN, :]
    if _trace:
        return out, res
    return out



# revision 5
# speedup vs baseline: 1.3956x; 1.3956x over previous
"""MeanField CRF message-passing kernel for 8 Trainium2 NeuronCores.

Sharding: (B=2) x (H into 4 chunks of 128 rows) = 8 slabs, each with a
5-row halo on slab-interior edges (5 mean-field iterations x 1-row
stencil reach), so cores run fully independently (no collectives).

Per-core layouts (all 16-bit except the fp32 softmax sums):
  x-layout  : [x mod 128 -> partitions, (xblock, class, y) -> free]
  C-packed  : [(class*6+row)=126 (+2 pad) -> partitions, x -> free]
Math per iteration (equivalent-transformed from the reference):
  E   = exp(-Y)                        (ACT, bf16, class-major blocks)
  s   = sum_c E ; r = 1/s              (gpsimd reduce f32 + DVE recip->bf16)
  m   = blockdiag(LC^T/8) @ E_C        (PE matmul; E_C via XBAR DMA transpose)
  w2_d= ew_d * shift_d(r)              (DVE bf16 2x, normalizer folded)
  Y   = u + sum_d w2_d * shift_d(m)    (DVE bf16 2x muls/tree; gpsimd adds)
All m-shifts come from XBAR DMA transposes of MC (x in the free dim), with
shifted OUTPUT windows per dy so no per-row edge fixups are needed.
Final cost = Y after iteration 5 (no softmax on the last iteration).
"""

import sys

sys.path.insert(0, "/opt/trn_rl_repo")

import numpy as np
import ml_dtypes

import concourse.bass as bass
import concourse.bacc as bacc
import concourse.tile as tile
from concourse import mybir
from concourse.bass_utils import run_bass_kernel_spmd

F32 = mybir.dt.float32
F16 = mybir.dt.float16
BF16 = mybir.dt.bfloat16

P = 128          # partitions
C = 21           # classes
RG = 6           # y-rows per packed group (21*6=126 of 128 partitions)
PK = 128         # padded packed-block size
NB = 23          # row-groups per slab (138 = 6*23)
YT = 138         # slab rows (128 own + 2*5 halo)
YTP = 140        # padded rows (1 pad row each end)
XB = 4           # x blocks (512 = 4*128)
D = 8            # directions
W = 512
HALO = 5
OWN = 128
MAX_ITER = 5
DIRS = [(0, 1), (0, -1), (1, 0), (-1, 0), (1, 1), (1, -1), (-1, 1), (-1, -1)]
# (dy, [(dir index, dx), ...])
GROUPS = [
    (0, [(0, 1), (1, -1)]),
    (1, [(2, 0), (4, 1), (5, -1)]),
    (-1, [(3, 0), (6, 1), (7, -1)]),
]
WINDOWS = [(0, 6), (6, 6), (12, 6), (18, 5)]  # (first group, n groups)

_CACHED_NC = None

ADD = mybir.AluOpType.add


def build_nc():
    nc = bacc.Bacc("TRN2")
    uu_d = nc.dram_tensor("uu", [P, XB, C, YTP], BF16, kind="ExternalInput")
    ew_d = nc.dram_tensor("ew", [P, D, XB, YT], F16, kind="ExternalInput")
    lcb_d = nc.dram_tensor("lcblk", [C * RG, C * RG], BF16, kind="ExternalInput")
    yout_d = nc.dram_tensor("yout", [P, XB, C, YTP], BF16, kind="ExternalOutput")

    with tile.TileContext(nc) as tc:
        with (
            tc.tile_pool(name="state", bufs=1) as st,
            tc.tile_pool(name="mxp", bufs=2) as mxp,
            tc.tile_pool(name="tp", bufs=4) as tp,
            tc.tile_pool(name="gp", bufs=8) as gp,
            tc.tile_pool(name="pm", bufs=3, space="PSUM") as pm,
        ):
            UU = st.tile([P, XB, C, YTP], BF16)
            Y = st.tile([P, XB, C, YTP], BF16)
            EB = st.tile([P, XB, NB, PK], BF16)   # exp(-Y), class-major blocks
            EC = st.tile([P, XB, NB, PK], BF16)   # transposed E (C-packed)
            MC = st.tile([P, NB, W + 2], BF16)    # m, x-padded, C-packed
            EWs = st.tile([P, D, XB, YT], F16)
            W2P = st.tile([P, D, XB, YTP], BF16)
            S0 = st.tile([P, XB, YT], F32)
            S0B = st.tile([P, XB, YT], BF16)
            RP = st.tile([P, XB, YT], BF16)
            RM = st.tile([P, XB, YT], BF16)
            LCB = st.tile([C * RG, C * RG], BF16)

            nc.sync.dma_start(out=UU[:], in_=uu_d[:])
            nc.sync.dma_start(out=EWs[:], in_=ew_d[:])
            nc.sync.dma_start(out=LCB[:], in_=lcb_d[:])
            nc.vector.memset(Y[:], 0)
            nc.vector.memset(EB[:], 0)
            nc.vector.memset(MC[:], 0)
            nc.vector.memset(W2P[:], 0)
            nc.vector.memset(RP[:], 0)
            nc.vector.memset(RM[:], 0)

            for it in range(MAX_ITER):
                # ---- E = exp(-Y) into class-major packed blocks ----------
                for xb in range(XB):
                    ev = EB[:, xb, :, 0:126].rearrange(
                        "p b (c r) -> p c b r", c=C, r=RG
                    )
                    if it == 0:
                        yv = UU[:, xb, :, 1 : 1 + YT]
                    else:
                        yv = Y[:, xb, :, 1 : 1 + YT]
                    yv = yv.rearrange("p c (b r) -> p c b r", b=NB, r=RG)
                    nc.scalar.activation(
                        out=ev, in_=yv,
                        func=mybir.ActivationFunctionType.Exp, scale=-1.0,
                    )
                # ---- softmax sums + reciprocal -> bf16 -------------------
                for xb in range(XB):
                    rin = EB[:, xb, :, 0:126].rearrange(
                        "p b (c r) -> p b r c", c=C, r=RG
                    )
                    rout = S0[:, xb, :].rearrange("p (b r) -> p b r", b=NB, r=RG)
                    nc.vector.tensor_reduce(
                        out=rout, in_=rin, op=ADD, axis=mybir.AxisListType.X
                    )
                with nc.allow_low_precision(reason="r multiplies bf16 q anyway"):
                    nc.vector.reciprocal(out=S0B[:], in_=S0[:])
                # ---- x+-1 shifted copies of r ----------------------------
                nc.sync.dma_start(out=RP[0 : P - 1, :, :], in_=S0B[1:P, :, :])
                nc.sync.dma_start(
                    out=RP[P - 1 : P, 0 : XB - 1, :], in_=S0B[0:1, 1:XB, :]
                )
                nc.sync.dma_start(out=RM[1:P, :, :], in_=S0B[0 : P - 1, :, :])
                nc.sync.dma_start(
                    out=RM[0:1, 1:XB, :], in_=S0B[P - 1 : P, 0 : XB - 1, :]
                )
                # ---- w2_d = ew_d * shift_d(r)  (bf16 2x) -----------------
                for d, (dy, dx) in enumerate(DIRS):
                    rsrc = {-1: RM, 0: S0B, 1: RP}[dx]
                    ylo, yhi = max(0, -dy), min(YT, YT - dy)
                    for xb in range(XB):
                        nc.vector.tensor_mul(
                            out=W2P[:, d, xb, 1 + ylo : 1 + yhi],
                            in0=EWs[:, d, xb, ylo:yhi],
                            in1=rsrc[:, xb, ylo + dy : yhi + dy],
                        )
                # ---- phase A: XBAR transpose E, LC matmul, evacuate ------
                for xb in range(XB):
                    nc.sync.dma_start_transpose(
                        out=EC[:, xb, :, :],
                        in_=EB[:, xb, :, :].rearrange("p a b -> p (a b)"),
                    )
                for rb in range(NB):
                    mcp = pm.tile([126, W], F32, tag="mcp")
                    nc.tensor.matmul(
                        out=mcp[:],
                        lhsT=LCB[:],
                        rhs=EC[0:126, :, rb, :],
                        start=True, stop=True,
                    )
                    nc.scalar.copy(out=MC[0:126, rb, 1 : W + 1], in_=mcp[:])

                # ---- phase B: XBAR shift transposes + accumulation -------
                pending = []  # deferred dy=-1 adds from previous window
                for w, (g0, ng) in enumerate(WINDOWS):
                    y0 = g0 * RG
                    n = ng * RG
                    mxt = {}
                    for dx in (-1, 0, 1):
                        t_mx = mxp.tile(
                            [P, XB, 6, PK], BF16, tag=f"mx{dx + 1}"
                        )
                        mxt[dx] = t_mx
                    for j in range(ng):
                        rbg = g0 + j
                        for dx in (-1, 0, 1):
                            nc.sync.dma_start_transpose(
                                out=mxt[dx][:, :, j, :],
                                in_=MC[:, rbg, dx + 1 : dx + 1 + W],
                            )
                    adds = []  # (dy, xb, z0, g tile)
                    for xb in range(XB):
                        for dy, dirs_g in GROUPS:
                            z0 = y0 - dy + 1
                            ts = []
                            for d, dx in dirs_g:
                                tag = "gm1" if (dy == -1 and not ts) else "tt"
                                pool = gp if tag == "gm1" else tp
                                t = pool.tile([P, 6 * 126], BF16, tag=tag)
                                tv = t[:, 0 : ng * 126].rearrange(
                                    "p (g c r) -> p g c r", g=ng, c=C, r=RG
                                )
                                mxv = mxt[dx][:, xb, 0:ng, 0:126].rearrange(
                                    "p g (c r) -> p g c r", c=C, r=RG
                                )
                                w2v = (
                                    W2P[:, d, xb, z0 : z0 + n]
                                    .rearrange(
                                        "p (g c r) -> p g c r", g=ng, c=1, r=RG
                                    )
                                    .to_broadcast((P, ng, C, RG))
                                )
                                nc.vector.tensor_mul(out=tv, in0=mxv, in1=w2v)
                                ts.append(t)
                            g = ts[0]
                            for t2 in ts[1:]:
                                nc.vector.tensor_add(
                                    out=g[:, 0 : ng * 126],
                                    in0=g[:, 0 : ng * 126],
                                    in1=t2[:, 0 : ng * 126],
                                )
                            adds.append((dy, xb, z0, ng, g))
                    # ordered Y updates on gpsimd: dy=0 (init from UU),
                    # dy=+1, then the previous window's deferred dy=-1.
                    def yadd(dy, xb, z0, ngg, g, first):
                        nn = ngg * RG
                        yv = Y[:, xb, :, z0 : z0 + nn].rearrange(
                            "p c (g r) -> p g c r", g=ngg, r=RG
                        )
                        src = UU if first else Y
                        sv = src[:, xb, :, z0 : z0 + nn].rearrange(
                            "p c (g r) -> p g c r", g=ngg, r=RG
                        )
                        gv = g[:, 0 : ngg * 126].rearrange(
                            "p (g c r) -> p g c r", g=ngg, c=C, r=RG
                        )
                        nc.gpsimd.tensor_add(out=yv, in0=sv, in1=gv)

                    for dy, xb, z0, ngg, g in adds:
                        if dy == 0:
                            yadd(dy, xb, z0, ngg, g, True)
                    for dy, xb, z0, ngg, g in adds:
                        if dy == 1:
                            yadd(dy, xb, z0, ngg, g, False)
                    for dy, xb, z0, ngg, g in pending:
                        yadd(dy, xb, z0, ngg, g, False)
                    pending = [a for a in adds if a[0] == -1]
                    if w == len(WINDOWS) - 1:
                        for dy, xb, z0, ngg, g in pending:
                            yadd(dy, xb, z0, ngg, g, False)
                        pending = []

            nc.sync.dma_start(out=yout_d[:], in_=Y[:])

    nc.finalize()
    return nc


def _prep_core(u, ew, b, hc):
    y0 = 128 * hc
    ys = min(max(y0 - HALO, 0), 512 - YT)
    u_slab = u[b, 0, :, ys : ys + YT, :]          # [21, 138, 512]
    ew_slab = ew[b, :, ys : ys + YT, :]           # [8, 138, 512]
    uu = np.zeros((P, XB, C, YTP), np.float32)
    uu[:, :, :, 1 : 1 + YT] = u_slab.reshape(C, YT, XB, P).transpose(3, 2, 0, 1)
    ewp = np.ascontiguousarray(
        ew_slab.reshape(D, YT, XB, P).transpose(3, 0, 2, 1), dtype=np.float16
    )
    return uu.astype(ml_dtypes.bfloat16), ewp, ys, y0 - ys


def kernel(unary, edge_weights, label_context, _trace=False, _tmpdir=None):
    global _CACHED_NC
    if _CACHED_NC is None:
        _CACHED_NC = build_nc()
    nc = _CACHED_NC

    u = np.asarray(unary, dtype=np.float32)
    ew = np.asarray(edge_weights, dtype=np.float32)
    lc = np.asarray(label_context, dtype=np.float32)

    lcblk = np.kron(lc.T / 8.0, np.eye(RG, dtype=np.float32)).astype(
        ml_dtypes.bfloat16
    )

    in_maps = []
    offs = []
    for core in range(8):
        b, hc = core // 4, core % 4
        uu, ewp, ys, off = _prep_core(u, ew, b, hc)
        offs.append(off)
        in_maps.append({"uu": uu, "ew": ewp, "lcblk": lcblk})

    kwargs = {}
    if _trace:
        kwargs = dict(trace=True, trace_cores=[0], tmpdir=_tmpdir)
    res = run_bass_kernel_spmd(nc, in_maps, core_ids=list(range(8)), **kwargs)

    out = np.zeros((2, 1, C, 512, 512), dtype=np.float32)
    for core in range(8):
        b, hc = core // 4, core % 4
        yo = np.asarray(res.results[core]["yout"], dtype=np.float32)
        slab = yo.transpose(2, 3, 1, 0).reshape(C, YTP, W)
        off = offs[core]
        out[b, 0, :, 128 * hc : 128 * (hc + 1), :] = slab[
            :, 1 + off : 1 + off + OWN, :
        ]
    if _trace:
        return out, res
    return out
